# revision 1
# baseline (speedup 1.0000x reference)
"""Trainium2 Bass kernel for a 4-head spatial MultiHeadAttention block.

Reference computation (per batch n):
    q/k/v = 1x1-conv projections of x (C=256 channels, S=48*48=2304 positions)
    per head (4 heads, d=64): attn = softmax(q^T k / 8), out = attn @ v
    out = Wo @ concat(heads) + bo + x   (residual)

Sharding across 8 NeuronCores: core c handles batch n = c//2 and head-pair
hp = c%2 (output channels [hp*128, hp*128+128) of the QKV projections, i.e.
heads {2*hp, 2*hp+1}).  Each core computes a partial output
Wo[:, ch] @ attn_ch (256 x 2304); the host sums the two partials per batch
and adds bo + residual x.

Per-core kernel layout choices:
  - Q stored (d, s), d on partitions: rows 0-63 head A, 64-127 head B.
  - K stored zero-padded per head (Kz0: head A rows + zero rows, Kz1: head B
    rows + zero rows) so every scores matmul contracts the full 128
    partitions and all attention matmuls share one PE tile config
    (128x128) - PE tile-config switches cost ~150ns per matmul.
  - V is produced transposed (VT: t on partitions, d on free) directly by the
    projection matmul, with a constant-1 column appended per head so the
    attn@V matmul also yields the softmax row-sums for free (M=65).
  - scoresT(t,s) = Kz_h^T Q; 3 t-tiles are packed into one 3-bank PSUM tile
    so exp (ScalarE) runs on 1536-wide batches straight out of PSUM.
  - software pipeline: the attn@V matmuls of exp-batch g are emitted after
    the scores matmuls of batch g+1, so the PE never waits on ScalarE.
  - normalization: reciprocal on a (64, sw/64) lane-spread reshape (a plain
    (1, sw) reciprocal runs on a single DVE lane at 8 cycles/element), then
    partition-broadcast via a DRAM bounce.
All matmul operands are bf16; accumulation and softmax math are fp32.
"""

import numpy as np

import concourse.bass as bass
import concourse.mybir as mybir
import concourse.tile as tile
from concourse import bacc
from concourse.bass_utils import run_bass_kernel_spmd

C = 256          # channels
S = 2304         # spatial positions (48*48)
HD = 64          # head dim
P = 128          # partitions
TT = S // P      # 18 t-tiles of 128
GRP = 3          # t-tiles per exp batch (3 PSUM banks)
SCALE = 0.125    # 1/sqrt(HD)
F32 = mybir.dt.float32
BF16 = mybir.dt.bfloat16

S_CHUNKS = [(0, 512), (512, 512), (1024, 512), (1536, 512), (2048, 256)]


def _body(tc):
    nc = tc.nc
    t_x = nc.dram_tensor("x", [C, S], BF16, kind="ExternalInput").ap()
    t_wqt = nc.dram_tensor("wqt", [C, P], BF16, kind="ExternalInput").ap()
    t_wkt = nc.dram_tensor("wkt", [C, P], BF16, kind="ExternalInput").ap()
    t_wvt = nc.dram_tensor("wvt", [C, P], BF16, kind="ExternalInput").ap()
    t_wot = nc.dram_tensor("wot", [P, C], BF16, kind="ExternalInput").ap()
    t_bq = nc.dram_tensor("bq", [P, 1], F32, kind="ExternalInput").ap()
    t_bk = nc.dram_tensor("bk", [P, 1], F32, kind="ExternalInput").ap()
    t_bv = nc.dram_tensor("bv", [1, P], F32, kind="ExternalInput").ap()
    t_out = nc.dram_tensor("out", [C, S], F32, kind="ExternalOutput").ap()

    singles = tc.alloc_tile_pool(name="singles", bufs=1)
    x_lo = singles.tile([P, S], BF16)
    x_hi = singles.tile([P, S], BF16)
    q_sb = singles.tile([P, S], BF16)
    kz0 = singles.tile([P, S], BF16)          # head A rows 0-63, zeros 64-127
    kz1 = singles.tile([P, S], BF16)          # zeros 0-63, head B rows 64-127
    vt_sb = singles.tile([P, TT, 130], BF16)  # per tt: [dA(64) | 1 | dB(64) | 1]
    wq_sb = singles.tile([P, 2, P], BF16)
    wk_sb = singles.tile([P, 2, P], BF16)
    wv_sb = singles.tile([P, 2, P], BF16)
    wot_sb = singles.tile([P, C], BF16)
    attn_full = singles.tile([P, S], BF16)
    bq_sb = singles.tile([P, 1], F32)
    bk_sb = singles.tile([P, 1], F32)
    bv_bc = singles.tile([P, P], F32)

    # ---- input DMAs: weights first (tiny), then x split across two queues ----
    nc.sync.dma_start(out=wk_sb, in_=t_wkt.rearrange("(a p) d -> p a d", p=P))
    nc.sync.dma_start(out=x_lo[:, 0:512], in_=t_x[0:P, 0:512])
    nc.sync.dma_start(out=x_hi[:, 0:512], in_=t_x[P:C, 0:512])
    nc.gpsimd.dma_start(out=bk_sb, in_=t_bk)
    nc.gpsimd.dma_start(out=bq_sb, in_=t_bq)
    nc.sync.dma_start(out=wq_sb, in_=t_wqt.rearrange("(a p) d -> p a d", p=P))
    nc.gpsimd.dma_start(out=wv_sb, in_=t_wvt.rearrange("(a p) d -> p a d", p=P))
    nc.gpsimd.dma_start(out=bv_bc, in_=t_bv.to_broadcast([P, P]))
    nc.gpsimd.dma_start(out=wot_sb, in_=t_wot)
    for ci, (s0, sw) in enumerate(S_CHUNKS[1:]):
        eng = nc.sync if ci % 2 == 0 else nc.gpsimd
        eng.dma_start(out=x_lo[:, s0:s0 + sw], in_=t_x[0:P, s0:s0 + sw])
        eng.dma_start(out=x_hi[:, s0:s0 + sw], in_=t_x[P:C, s0:s0 + sw])
    # zero the dead half of each Kz; ones-columns (64, 129) of vt survive the
    # per-tile evictions which overwrite all other columns
    nc.vector.memset(kz0[HD:P, :], 0.0)
    nc.vector.memset(kz1[0:HD, :], 0.0)
    nc.vector.memset(vt_sb[:, :, :], 1.0)

    ps = tc.alloc_tile_pool(name="ps", bufs=2, space="PSUM")
    ex_pool = tc.alloc_tile_pool(name="ex_sb", bufs=4)
    nrm = tc.alloc_tile_pool(name="nrm", bufs=2)
    wo_out = tc.alloc_tile_pool(name="wo_out", bufs=4)
    sdram = tc.alloc_tile_pool(name="sdram", bufs=2, space="DRAM")

    def k_chunk(s0, sw):
        psn = ps.tile([P, GRP * 512], F32, tag="sc", name="kps")[:, :sw]
        nc.tensor.matmul(psn, wk_sb[:, 0, :], x_lo[:, s0:s0 + sw],
                         start=True, stop=False)
        nc.tensor.matmul(psn, wk_sb[:, 1, :], x_hi[:, s0:s0 + sw],
                         start=False, stop=True)
        nc.vector.tensor_scalar_add(kz0[0:HD, s0:s0 + sw], psn[0:HD, :],
                                    bk_sb[0:HD, :])
        nc.vector.tensor_scalar_add(kz1[HD:P, s0:s0 + sw], psn[HD:P, :],
                                    bk_sb[HD:P, :])

    def q_chunk(s0, sw):
        psn = ps.tile([P, GRP * 512], F32, tag="sc", name="qps")[:, :sw]
        nc.tensor.matmul(psn, wq_sb[:, 0, :], x_lo[:, s0:s0 + sw],
                         start=True, stop=False)
        nc.tensor.matmul(psn, wq_sb[:, 1, :], x_hi[:, s0:s0 + sw],
                         start=False, stop=True)
        nc.vector.tensor_scalar_add(q_sb[:, s0:s0 + sw], psn, bq_sb)

    def vt_tiles(tts):
        for tt in tts:
            psn = ps.tile([P, GRP * 512], F32, tag="sc", name="vtps")[:, :P]
            nc.tensor.matmul(psn, x_lo[:, tt * P:(tt + 1) * P], wv_sb[:, 0, :],
                             start=True, stop=False)
            nc.tensor.matmul(psn, x_hi[:, tt * P:(tt + 1) * P], wv_sb[:, 1, :],
                             start=False, stop=True)
            nc.vector.tensor_add(vt_sb[:, tt, 0:HD], psn[:, 0:HD], bv_bc[:, 0:HD])
            nc.vector.tensor_add(vt_sb[:, tt, 65:65 + HD], psn[:, HD:P],
                                 bv_bc[:, HD:P])

    def emit_av(pend):
        ex, g, ot, h, sw = pend
        for j in range(GRP):
            tt = g * GRP + j
            nc.tensor.matmul(ot, vt_sb[:, tt, h * 65:(h + 1) * 65],
                             ex[:, j * sw:(j + 1) * sw],
                             start=(tt == 0), stop=(tt == TT - 1))

    def wo_chunk(s0, sw):
        for half in range(2):
            psn = ps.tile([P, GRP * 512], F32, tag="sc", name="wops")[:, :sw]
            nc.tensor.matmul(psn, wot_sb[:, half * P:(half + 1) * P],
                             attn_full[:, s0:s0 + sw], start=True, stop=True)
            ob = wo_out.tile([P, 512], F32, tag="ob", name="ob")[:, :sw]
            nc.vector.tensor_copy(ob, psn)
            nc.sync.dma_start(out=t_out[half * P:(half + 1) * P, s0:s0 + sw],
                              in_=ob)

    def emit_norm(ot, h, s0, sw):
        comb = nrm.tile([65, 512], F32, tag="comb", name="comb")[:, :sw]
        nc.vector.tensor_copy(comb, ot)
        w8 = sw // HD  # elements per lane after the (64, w8) spread
        rs = nrm.tile([HD, 8], F32, tag="rs", name="rs")[:, :w8]
        nc.sync.dma_start(out=rs, in_=comb[HD:HD + 1, :])
        rr = nrm.tile([HD, 8], F32, tag="rr", name="rr")[:, :w8]
        nc.vector.reciprocal(rr, rs)
        lin = nrm.tile([1, 512], F32, tag="lin", name="lin")[:, :sw]
        nc.sync.dma_start(out=lin, in_=rr)
        rb = nrm.tile([HD, 512], F32, tag="rb", name="rb")[:, :sw]
        lin_bc = bass.AP(tensor=lin.tensor, offset=lin.offset,
                         ap=[lin.ap[0], [0, HD], lin.ap[1]])
        nc.sync.dma_start(out=rb, in_=lin_bc)
        if h == 0:
            nc.vector.tensor_mul(attn_full[0:HD, s0:s0 + sw], comb[0:HD, :], rb)
        else:
            a1 = nrm.tile([HD, 512], BF16, tag="a1", name="a1")[:, :sw]
            nc.vector.tensor_mul(a1, comb[0:HD, :], rb)
            nc.sync.dma_start(out=attn_full[HD:P, s0:s0 + sw], in_=a1)

    # ---- projections needed before the first exp batch ----
    for s0, sw in S_CHUNKS:
        k_chunk(s0, sw)
    q_chunk(*S_CHUNKS[0])

    # ---- attention: software-pipelined across all (s-chunk, head) units ----
    pend = None       # (ex, g, ot, h, sw): exp batch whose attn@V is pending
    pend_norm = None  # (ot, h, s0, sw): unit awaiting normalization
    wo_q = []         # (unit_idx, s0, sw) chunks whose Wo projection is pending
    weave = 0         # startup weave: VT + remaining Q between early exps
    unit = 0
    for s0, sw in S_CHUNKS:
        for h in range(2):
            unit += 1
            kz = kz0 if h == 0 else kz1
            ot = ps.tile([65, 512], F32, tag="ot", name="ot")[:, :sw]
            for g in range(TT // GRP):
                sc = ps.tile([P, GRP * 512], F32, tag="sc", name="sc")[:, :GRP * sw]
                for j in range(GRP):
                    tt = g * GRP + j
                    nc.tensor.matmul(sc[:, j * sw:(j + 1) * sw],
                                     kz[:, tt * P:(tt + 1) * P],
                                     q_sb[:, s0:s0 + sw],
                                     start=True, stop=True)
                # startup weave, fine-grained: VT tiles 3 per group, emitted
                # just before the attn@V batch that reads them (Tile deps are
                # emission-ordered); remaining Q chunks one per group after VT
                if weave < TT // GRP and (s0, h) == (0, 0) and g >= 1:
                    vt_tiles(range(weave * GRP, (weave + 1) * GRP))
                    weave += 1
                elif weave < TT // GRP:  # (s0, h1): finish VT
                    vt_tiles(range(weave * GRP, (weave + 1) * GRP))
                    weave += 1
                elif weave < TT // GRP + len(S_CHUNKS) - 1:
                    q_chunk(*S_CHUNKS[weave - TT // GRP + 1])
                    weave += 1
                if pend is not None:
                    emit_av(pend)
                    if pend[1] == TT // GRP - 1:  # last batch of its unit
                        emit_norm(*pend_norm)
                        if pend_norm[1] == 1:  # head B done: queue Wo
                            wo_q.append((unit, pend_norm[2], pend_norm[3]))
                if wo_q and g == 2 and unit > wo_q[0][0] + 1:
                    # a full unit after its normalize chain started
                    wo_chunk(*wo_q.pop(0)[1:])
                ex = ex_pool.tile([P, GRP * 512], BF16, tag="ex", name="ex")[:, :GRP * sw]
                nc.scalar.activation(ex, sc, mybir.ActivationFunctionType.Exp,
                                     scale=SCALE)
                pend = (ex, g, ot, h, sw)
                if g == TT // GRP - 1:
                    pend_norm = (ot, h, s0, sw)
    emit_av(pend)
    emit_norm(*pend_norm)
    wo_q.append((unit, pend_norm[2], pend_norm[3]))
    for wq_item in wo_q:
        wo_chunk(*wq_item[1:])

    sdram.release()
    wo_out.release()
    nrm.release()
    ex_pool.release()
    ps.release()
    singles.release()


_NC_CACHE = {}


def build_nc():
    if "nc" not in _NC_CACHE:
        nc = bacc.Bacc("TRN2", target_bir_lowering=False, debug=False, num_devices=8)
        with tile.TileContext(nc) as tc:
            _body(tc)
        nc.compile()
        _NC_CACHE["nc"] = nc
    return _NC_CACHE["nc"]


def make_in_maps(x, Wq, bq, Wk, bk, Wv, bv, Wo, bo):
    import ml_dtypes
    bf16 = ml_dtypes.bfloat16
    N = x.shape[0]
    xf = np.ascontiguousarray(np.asarray(x, np.float32).reshape(N, C, S).astype(bf16))
    in_maps = []
    for c in range(8):
        n, hp = c // 2, c % 2
        ch = slice(hp * P, (hp + 1) * P)
        wot = np.ascontiguousarray(np.asarray(Wo, np.float32)[:, ch].T.astype(bf16))  # (128, 256)
        in_maps.append({
            "x": xf[n],
            "wqt": np.ascontiguousarray(np.asarray(Wq, np.float32)[ch].T.astype(bf16)),
            "wkt": np.ascontiguousarray(np.asarray(Wk, np.float32)[ch].T.astype(bf16)),
            "wvt": np.ascontiguousarray(np.asarray(Wv, np.float32)[ch].T.astype(bf16)),
            "wot": wot,
            "bq": np.ascontiguousarray(np.asarray(bq, np.float32)[ch].reshape(P, 1)),
            "bk": np.ascontiguousarray(np.asarray(bk, np.float32)[ch].reshape(P, 1)),
            "bv": np.ascontiguousarray(np.asarray(bv, np.float32)[ch].reshape(1, P)),
        })
    return in_maps


def run(inputs, **kwargs):
    """Run on 8 cores; returns (full output, BassKernelResults)."""
    nc = build_nc()
    in_maps = make_in_maps(**inputs)
    res = run_bass_kernel_spmd(nc, in_maps, core_ids=list(range(8)), **kwargs)
    x = np.asarray(inputs["x"], np.float32)
    bo = np.asarray(inputs["bo"], np.float32)
    N, _, H, W = x.shape
    out = np.empty((N, C, S), np.float32)
    for n in range(N):
        out[n] = (x[n].reshape(C, S)
                  + res.results[2 * n]["out"]
                  + res.results[2 * n + 1]["out"]
                  + bo[:, None])
    return out.reshape(N, C, H, W), res


def kernel(**inputs):
    out, _ = run(inputs)
    return out



# revision 4
# speedup vs baseline: 1.1627x; 1.1627x over previous
"""Trainium2 Bass kernel for a 4-head spatial MultiHeadAttention block.

Reference computation (per batch n):
    q/k/v = 1x1-conv projections of x (C=256 channels, S=48*48=2304 positions)
    per head (4 heads, d=64): attn = softmax(q^T k / 8), out = attn @ v
    out = Wo @ concat(heads) + bo + x   (residual)

Sharding across 8 NeuronCores: core c handles batch n = c//2 and head-pair
hp = c%2 (output channels [hp*128, hp*128+128) of the QKV projections, i.e.
heads {2*hp, 2*hp+1}).  Each core computes a partial output
Wo[:, ch] @ attn_ch (256 x 2304); the host sums the two partials per batch
and adds bo + residual x.

Per-core kernel layout choices (v2):
  - All inputs pre-packed on the host partition-major so every DMA moves
    large contiguous per-partition rows (x lands in ~4us in 3 transfers).
  - DMA issues are spread across engine queues (sync/gpsimd/scalar/vector);
    SBUF memsets run on the otherwise-idle GpSimd engine.
  - ScalarE does ONLY exp (the ~87us serial floor); every bias add / copy
    runs on VectorE.  A 1-element dummy Exp pins the ACT table set early.
  - Q stored (d, s); K zero-padded per head (kz0/kz1) so every scores
    matmul contracts the full 128 partitions with one PE config.
  - VT produced transposed by the projection matmul with constant-1 columns
    so attn@V also yields softmax row-sums (M=65); VT/K-tail/Q-tail
    projections are emitted in large batched PSUM allocations woven into
    the first attention unit.
  - scoresT(t,s): 3 t-tiles per 3-bank PSUM group; exp (ScalarE) runs on
    1536-wide batches out of PSUM; attn@V of batch g is emitted after the
    scores of batch g+1 (software pipeline).
  - normalization: reciprocal_approx_fast on the (1,sw) row-sum row, then
    gpsimd.partition_broadcast to 64 lanes (no DMA bounce), DVE multiply.
  - Wo projection per finished chunk with a paired single PSUM alloc;
    bf16 output staged in SBUF and DMA'd out at chunk-pair boundaries.
All matmul operands are bf16; accumulation and softmax math are fp32.
"""

import numpy as np

import concourse.bass as bass
import concourse.mybir as mybir
import concourse.tile as tile
from concourse import bacc
from concourse.bass_utils import run_bass_kernel_spmd

C = 256          # channels
S = 2304         # spatial positions (48*48)
HD = 64          # head dim
P = 128          # partitions
TT = S // P      # 18 t-tiles of 128
GRP = 3          # t-tiles per exp batch (3 PSUM banks)
NG = TT // GRP   # 6 exp batches per unit
SCALE = 0.125    # 1/sqrt(HD)
F32 = mybir.dt.float32
BF16 = mybir.dt.bfloat16

S_CHUNKS = [(0, 512), (512, 512), (1024, 512), (1536, 512), (2048, 256)]
X_SPLITS = [(0, 1024), (1024, 1024), (2048, 256)]
# wqkv slot indices: [wq_a0, wq_a1, wk_a0, wk_a1, wv_a0, wv_a1]
WQ0, WQ1, WK0, WK1, WV0, WV1 = range(6)


def _body(tc):
    nc = tc.nc
    t_xx = nc.dram_tensor("xx", [P, 2, S], BF16, kind="ExternalInput").ap()
    t_wqkv = nc.dram_tensor("wqkv", [P, 6, P], BF16, kind="ExternalInput").ap()
    t_wot = nc.dram_tensor("wot", [P, C], BF16, kind="ExternalInput").ap()
    t_bq = nc.dram_tensor("bq", [P, 1], F32, kind="ExternalInput").ap()
    t_bk = nc.dram_tensor("bk", [P, 1], F32, kind="ExternalInput").ap()
    t_bv = nc.dram_tensor("bv", [P, P], F32, kind="ExternalInput").ap()
    t_out = nc.dram_tensor("out", [2, P, S], BF16, kind="ExternalOutput").ap()

    singles = tc.alloc_tile_pool(name="singles", bufs=1)
    xx = singles.tile([P, 2, S], BF16)
    q_sb = singles.tile([P, S], BF16)
    kz0 = singles.tile([P, S], BF16)          # head A rows 0-63, zeros 64-127
    kz1 = singles.tile([P, S], BF16)          # zeros 0-63, head B rows 64-127
    vt_sb = singles.tile([P, TT, 130], BF16)  # per tt: [dA(64) | 1 | dB(64) | 1]
    wqkv_sb = singles.tile([P, 6, P], BF16)
    wot_sb = singles.tile([P, C], BF16)
    attn_full = singles.tile([P, S], BF16)
    ob = singles.tile([P, 2, S], BF16)        # output staging [p, half, s]
    bq_sb = singles.tile([P, 1], F32)
    bk_sb = singles.tile([P, 1], F32)
    bv_bc = singles.tile([P, P], F32)
    scr = singles.tile([1, 1], F32)

    # ---- input DMAs: spread across engine queues, x first in 3 big pieces ----
    for s0, sw in X_SPLITS:
        nc.sync.dma_start(out=xx[:, :, s0:s0 + sw], in_=t_xx[:, :, s0:s0 + sw])
    nc.gpsimd.dma_start(out=wqkv_sb, in_=t_wqkv)
    nc.scalar.dma_start(out=bk_sb, in_=t_bk)
    nc.scalar.dma_start(out=bq_sb, in_=t_bq)
    nc.scalar.dma_start(out=wot_sb, in_=t_wot)
    nc.gpsimd.dma_start(out=bv_bc, in_=t_bv)
    # dead K halves + VT ones-columns; GpSimd is idle at startup
    nc.gpsimd.memset(kz0[HD:P, :], 0.0)
    nc.gpsimd.memset(kz1[0:HD, :], 0.0)
    nc.gpsimd.memset(vt_sb[:, :, HD:HD + 1], 1.0)
    nc.gpsimd.memset(vt_sb[:, :, 129:130], 1.0)
    # pin the exp table set before the first real exp
    nc.scalar.activation(scr, bk_sb[0:1, :], mybir.ActivationFunctionType.Exp)

    ps = tc.alloc_tile_pool(name="ps", bufs=2, space="PSUM")
    ex_pool = tc.alloc_tile_pool(name="ex_sb", bufs=4)
    nrm = tc.alloc_tile_pool(name="nrm", bufs=2)

    def k_chunk(ci):
        s0, sw = S_CHUNKS[ci]
        psn = ps.tile([P, GRP * 512], F32, tag="sc", name="kps")[:, :sw]
        nc.tensor.matmul(psn, wqkv_sb[:, WK0, :], xx[:, 0, s0:s0 + sw],
                         start=True, stop=False)
        nc.tensor.matmul(psn, wqkv_sb[:, WK1, :], xx[:, 1, s0:s0 + sw],
                         start=False, stop=True)
        nc.vector.tensor_scalar_add(kz0[0:HD, s0:s0 + sw], psn[0:HD, :],
                                    bk_sb[0:HD, :])
        nc.vector.tensor_scalar_add(kz1[HD:P, s0:s0 + sw], psn[HD:P, :],
                                    bk_sb[HD:P, :])

    def k_tail():
        # chunks 2..4 (s 1024:2304, 1280 wide) in one PSUM alloc, one add pair
        psn = ps.tile([P, GRP * 512], F32, tag="sc", name="kps")[:, :1280]
        for ci in (2, 3, 4):
            s0, sw = S_CHUNKS[ci]
            o = s0 - 1024
            nc.tensor.matmul(psn[:, o:o + sw], wqkv_sb[:, WK0, :],
                             xx[:, 0, s0:s0 + sw], start=True, stop=False)
            nc.tensor.matmul(psn[:, o:o + sw], wqkv_sb[:, WK1, :],
                             xx[:, 1, s0:s0 + sw], start=False, stop=True)
        nc.vector.tensor_scalar_add(kz0[0:HD, 1024:S], psn[0:HD, :],
                                    bk_sb[0:HD, :])
        nc.vector.tensor_scalar_add(kz1[HD:P, 1024:S], psn[HD:P, :],
                                    bk_sb[HD:P, :])

    def q_multi(cis):
        s0 = S_CHUNKS[cis[0]][0]
        wtot = sum(S_CHUNKS[ci][1] for ci in cis)
        psn = ps.tile([P, GRP * 512], F32, tag="sc", name="qps")[:, :wtot]
        for ci in cis:
            c0, cw = S_CHUNKS[ci]
            o = c0 - s0
            nc.tensor.matmul(psn[:, o:o + cw], wqkv_sb[:, WQ0, :],
                             xx[:, 0, c0:c0 + cw], start=True, stop=False)
            nc.tensor.matmul(psn[:, o:o + cw], wqkv_sb[:, WQ1, :],
                             xx[:, 1, c0:c0 + cw], start=False, stop=True)
        nc.vector.tensor_scalar_add(q_sb[:, s0:s0 + wtot], psn, bq_sb)

    def vt_multi(base, n):
        # n consecutive t-tiles in one PSUM alloc, one grouped bias add
        psn = ps.tile([P, GRP * 512], F32, tag="sc", name="vtps")
        ps3 = psn[:, :n * P].rearrange("p (n d) -> p n d", d=P)
        for j in range(n):
            tt = base + j
            nc.tensor.matmul(ps3[:, j, :], xx[:, 0, tt * P:(tt + 1) * P],
                             wqkv_sb[:, WV0, :], start=True, stop=False)
            nc.tensor.matmul(ps3[:, j, :], xx[:, 1, tt * P:(tt + 1) * P],
                             wqkv_sb[:, WV1, :], start=False, stop=True)
        # vt cols per tt: [dA 0:64 | one | dB 65:129 | one]; write both halves
        va = vt_sb[:, base:base + n, 0:HD]
        vb = vt_sb[:, base:base + n, HD + 1:129]
        pa = bass.AP(tensor=ps3.tensor, offset=ps3.offset,
                     ap=[ps3.ap[0], ps3.ap[1], [ps3.ap[2][0], HD]])
        pb_src = ps3[:, :, HD:P]
        bva = bass.AP(tensor=bv_bc.tensor, offset=bv_bc.offset,
                      ap=[bv_bc.ap[0], [0, n], [bv_bc.ap[1][0], HD]])
        bvb_base = bv_bc[:, HD:P]
        bvb = bass.AP(tensor=bvb_base.tensor, offset=bvb_base.offset,
                      ap=[bvb_base.ap[0], [0, n], bvb_base.ap[1]])
        nc.vector.tensor_add(va, pa, bva)
        nc.vector.tensor_add(vb, pb_src, bvb)

    def emit_av(pend):
        ex, g, ot, h, sw = pend
        for j in range(GRP):
            tt = g * GRP + j
            nc.tensor.matmul(ot, vt_sb[:, tt, h * 65:(h + 1) * 65],
                             ex[:, j * sw:(j + 1) * sw],
                             start=(tt == 0), stop=(tt == TT - 1))

    def wo_chunk(ci):
        s0, sw = S_CHUNKS[ci]
        psn = ps.tile([P, GRP * 512], F32, tag="sc", name="wops")
        for half in range(2):
            pw = psn[:, half * 512:half * 512 + sw]
            nc.tensor.matmul(pw, wot_sb[:, half * P:(half + 1) * P],
                             attn_full[:, s0:s0 + sw], start=True, stop=True)
            nc.vector.tensor_copy(ob[:, half, s0:s0 + sw], pw)

    def out_dma(s0, sw):
        for half in range(2):
            eng = nc.sync if half == 0 else nc.gpsimd
            eng.dma_start(out=t_out[half, :, s0:s0 + sw],
                          in_=ob[:, half, s0:s0 + sw])

    def emit_norm(ot, h, s0, sw):
        comb = nrm.tile([65, 512], F32, tag="comb", name="comb")[:, :sw]
        nc.vector.tensor_copy(comb, ot)
        # row-sum lives on partition 64; recip/broadcast need base partition 0
        rs0 = nrm.tile([1, 512], F32, tag="rs0", name="rs0")[:, :sw]
        nc.sync.dma_start(out=rs0, in_=comb[HD:HD + 1, :])
        rinv = nrm.tile([1, 512], F32, tag="rinv", name="rinv")[:, :sw]
        nc.vector.reciprocal_approx_fast(rinv, rs0)
        rb = nrm.tile([HD, 512], F32, tag="rb", name="rb")[:, :sw]
        nc.gpsimd.partition_broadcast(rb, rinv)
        if h == 0:
            nc.vector.tensor_mul(attn_full[0:HD, s0:s0 + sw], comb[0:HD, :], rb)
        else:
            a1 = nrm.tile([HD, 512], BF16, tag="a1", name="a1")[:, :sw]
            nc.vector.tensor_mul(a1, comb[0:HD, :], rb)
            nc.sync.dma_start(out=attn_full[HD:P, s0:s0 + sw], in_=a1)

    # ---- prologue compute: first two K chunks + Q chunk 0 ----
    k_chunk(0)
    q_multi([0])
    k_chunk(1)

    # ---- attention: software-pipelined across all (s-chunk, head) units ----
    pend = None       # (ex, g, ot, h, sw): exp batch whose attn@V is pending
    pend_norm = None  # (ot, h, s0, sw, ci): unit awaiting normalization
    wo_q = []         # (unit_idx, ci) chunks whose Wo projection is pending
    unit = 0
    for ci, (s0, sw) in enumerate(S_CHUNKS):
        for h in range(2):
            unit += 1
            kz = kz0 if h == 0 else kz1
            ot = ps.tile([65, 512], F32, tag="ot", name="ot")[:, :sw]
            for g in range(NG):
                if unit == 1 and g == 2:
                    k_tail()  # before scores g2 (t 768:1152 needs chunk 2)
                sc = ps.tile([P, GRP * 512], F32, tag="sc", name="sc")[:, :GRP * sw]
                for j in range(GRP):
                    tt = g * GRP + j
                    nc.tensor.matmul(sc[:, j * sw:(j + 1) * sw],
                                     kz[:, tt * P:(tt + 1) * P],
                                     q_sb[:, s0:s0 + sw],
                                     start=True, stop=True)
                # startup weave: VT + remaining Q between early exp batches;
                # VT batch b lands just before the attn@V that reads it
                if unit == 1:
                    if g == 1:
                        vt_multi(0, 9)
                    elif g == 3:
                        vt_multi(9, 9)
                    elif g == 4:
                        q_multi([1, 2])
                    elif g == 5:
                        q_multi([3, 4])
                if pend is not None:
                    emit_av(pend)
                    if pend[1] == NG - 1:  # last batch of its unit
                        emit_norm(*pend_norm[:4])
                        if pend_norm[1] == 1:  # head B done: queue Wo
                            wo_q.append((unit, pend_norm[4]))
                if wo_q and g == 2 and unit > wo_q[0][0]:
                    ci_w = wo_q.pop(0)[1]
                    wo_chunk(ci_w)
                    if ci_w == 1:
                        out_dma(0, 1024)
                    elif ci_w == 3:
                        out_dma(1024, 1024)
                ex = ex_pool.tile([P, GRP * 512], BF16, tag="ex", name="ex")[:, :GRP * sw]
                nc.scalar.activation(ex, sc, mybir.ActivationFunctionType.Exp,
                                     scale=SCALE)
                pend = (ex, g, ot, h, sw)
                if g == NG - 1:
                    pend_norm = (ot, h, s0, sw, ci)
    emit_av(pend)
    emit_norm(*pend_norm[:4])
    wo_q.append((unit, pend_norm[4]))
    for _, ci_w in wo_q:
        wo_chunk(ci_w)
        if ci_w == 1:
            out_dma(0, 1024)
        elif ci_w == 3:
            out_dma(1024, 1024)
    out_dma(2048, 256)

    nrm.release()
    ex_pool.release()
    ps.release()
    singles.release()


_NC_CACHE = {}


def build_nc():
    if "nc" not in _NC_CACHE:
        nc = bacc.Bacc("TRN2", target_bir_lowering=False, debug=False, num_devices=8)
        with tile.TileContext(nc) as tc:
            _body(tc)
        nc.compile()
        _NC_CACHE["nc"] = nc
    return _NC_CACHE["nc"]


def make_in_maps(x, Wq, bq, Wk, bk, Wv, bv, Wo, bo):
    import ml_dtypes
    bf16 = ml_dtypes.bfloat16
    N = x.shape[0]
    # (N, C, S) -> per batch (P, 2, S): partition p holds rows p and p+128
    xf = np.asarray(x, np.float32).reshape(N, C, S).reshape(N, 2, P, S)
    xf = np.ascontiguousarray(xf.transpose(0, 2, 1, 3).astype(bf16))
    in_maps = []
    for c in range(8):
        n, hp = c // 2, c % 2
        ch = slice(hp * P, (hp + 1) * P)
        wqkv = np.empty((P, 6, P), np.float32)
        for i, W in enumerate((Wq, Wk, Wv)):
            wt = np.asarray(W, np.float32)[ch].T  # (C, 128): [c_in, d_out]
            wqkv[:, 2 * i, :] = wt[0:P]
            wqkv[:, 2 * i + 1, :] = wt[P:C]
        wot = np.asarray(Wo, np.float32)[:, ch].T  # (128, 256)
        bvv = np.asarray(bv, np.float32)[ch]
        in_maps.append({
            "xx": xf[n],
            "wqkv": np.ascontiguousarray(wqkv.astype(bf16)),
            "wot": np.ascontiguousarray(wot.astype(bf16)),
            "bq": np.ascontiguousarray(np.asarray(bq, np.float32)[ch].reshape(P, 1)),
            "bk": np.ascontiguousarray(np.asarray(bk, np.float32)[ch].reshape(P, 1)),
            "bv": np.ascontiguousarray(np.broadcast_to(bvv[None, :], (P, P))),
        })
    return in_maps


def run(inputs, **kwargs):
    """Run on 8 cores; returns (full output, BassKernelResults)."""
    nc = build_nc()
    in_maps = make_in_maps(**inputs)
    res = run_bass_kernel_spmd(nc, in_maps, core_ids=list(range(8)), **kwargs)
    x = np.asarray(inputs["x"], np.float32)
    bo = np.asarray(inputs["bo"], np.float32)
    N, _, H, W = x.shape
    out = np.empty((N, C, S), np.float32)
    for n in range(N):
        p0 = np.asarray(res.results[2 * n]["out"], np.float32).reshape(C, S)
        p1 = np.asarray(res.results[2 * n + 1]["out"], np.float32).reshape(C, S)
        out[n] = x[n].reshape(C, S) + p0 + p1 + bo[:, None]
    return out.reshape(N, C, H, W), res


def kernel(**inputs):
    out, _ = run(inputs)
    return out


# revision 7
# speedup vs baseline: 1.2147x; 1.0448x over previous
"""Trainium2 Bass kernel for a 4-head spatial MultiHeadAttention block.

Reference computation (per batch n):
    q/k/v = 1x1-conv projections of x (C=256 channels, S=48*48=2304 positions)
    per head (4 heads, d=64): attn = softmax(q^T k / 8), out = attn @ v
    out = Wo @ concat(heads) + bo + x   (residual)

Sharding across 8 NeuronCores: core c handles batch n = c//2 and head-pair
hp = c%2 (output channels [hp*128, hp*128+128) of the QKV projections, i.e.
heads {2*hp, 2*hp+1}).  Each core computes a partial output
Wo[:, ch] @ attn_ch (256 x 2304); the host sums the two partials per batch
and adds bo + residual x.

Per-core kernel layout choices (v2):
  - All inputs pre-packed on the host partition-major so every DMA moves
    large contiguous per-partition rows (x lands in ~4us in 3 transfers).
  - DMA issues are spread across engine queues (sync/gpsimd/scalar/vector);
    SBUF memsets run on the otherwise-idle GpSimd engine.
  - ScalarE does ONLY exp (the ~87us serial floor); every bias add / copy
    runs on VectorE.  A 1-element dummy Exp pins the ACT table set early.
  - Q stored (d, s); K zero-padded per head (kz0/kz1) so every scores
    matmul contracts the full 128 partitions with one PE config.
  - VT produced transposed by the projection matmul with constant-1 columns
    so attn@V also yields softmax row-sums (M=65); VT/K-tail/Q-tail
    projections are emitted in large batched PSUM allocations woven into
    the first attention unit.
  - scoresT(t,s): 3 t-tiles per 3-bank PSUM group; exp (ScalarE) runs on
    1536-wide batches out of PSUM; attn@V of batch g is emitted after the
    scores of batch g+1 (software pipeline).
  - normalization: reciprocal_approx_fast on the (1,sw) row-sum row, then
    gpsimd.partition_broadcast to 64 lanes (no DMA bounce), DVE multiply.
  - Wo projection per finished chunk with a paired single PSUM alloc;
    bf16 output staged in SBUF and DMA'd out at chunk-pair boundaries.
All matmul operands are bf16; accumulation and softmax math are fp32.
"""

import numpy as np

import concourse.bass as bass
import concourse.mybir as mybir
import concourse.tile as tile
from concourse import bacc
from concourse.bass_utils import run_bass_kernel_spmd

C = 256          # channels
S = 2304         # spatial positions (48*48)
HD = 64          # head dim
P = 128          # partitions
TT = S // P      # 18 t-tiles of 128
GRP = 3          # t-tiles per exp batch (3 PSUM banks)
NG = TT // GRP   # 6 exp batches per unit
SCALE = 0.125    # 1/sqrt(HD)
F32 = mybir.dt.float32
BF16 = mybir.dt.bfloat16

S_CHUNKS = [(0, 512), (512, 512), (1024, 512), (1536, 512), (2048, 256)]
X_SPLITS = [(0, 512), (512, 1792)]
# wqkv slot indices: [wq_a0, wq_a1, wk_a0, wk_a1, wv_a0, wv_a1]
WQ0, WQ1, WK0, WK1, WV0, WV1 = range(6)


def _body(tc):
    nc = tc.nc
    t_xx = nc.dram_tensor("xx", [P, 2, S], BF16, kind="ExternalInput").ap()
    t_wqkv = nc.dram_tensor("wqkv", [P, 6, P], BF16, kind="ExternalInput").ap()
    t_wot = nc.dram_tensor("wot", [P, C], BF16, kind="ExternalInput").ap()
    t_bq = nc.dram_tensor("bq", [P, 1], F32, kind="ExternalInput").ap()
    t_bk = nc.dram_tensor("bk", [P, 1], F32, kind="ExternalInput").ap()
    t_bv = nc.dram_tensor("bv", [P, P], F32, kind="ExternalInput").ap()
    t_out = nc.dram_tensor("out", [2, P, S], BF16, kind="ExternalOutput").ap()

    singles = tc.alloc_tile_pool(name="singles", bufs=1)
    xx = singles.tile([P, 2, S], BF16)
    q_sb = singles.tile([P, S], BF16)
    kz0 = singles.tile([P, S], BF16)          # head A rows 0-63, zeros 64-127
    kz1 = singles.tile([P, S], BF16)          # zeros 0-63, head B rows 64-127
    vt_sb = singles.tile([P, TT, 130], BF16)  # per tt: [dA(64) | 1 | dB(64) | 1]
    wqkv_sb = singles.tile([P, 6, P], BF16)
    wot_sb = singles.tile([P, C], BF16)
    attn_full = singles.tile([P, S], BF16)
    ob = singles.tile([P, 2, S], BF16)        # output staging [p, half, s]
    bq_sb = singles.tile([P, 1], F32)
    bk_sb = singles.tile([P, 1], F32)
    bv_bc = singles.tile([P, P], F32)
    scr = singles.tile([1, 1], F32)

    # ---- input DMAs: small weights first so they beat x through the queues,
    # then x in two pieces with large per-partition descriptors ----
    nc.gpsimd.dma_start(out=wqkv_sb, in_=t_wqkv)
    nc.scalar.dma_start(out=bk_sb, in_=t_bk)
    nc.scalar.dma_start(out=bq_sb, in_=t_bq)
    nc.gpsimd.dma_start(out=bv_bc, in_=t_bv)
    for s0, sw in X_SPLITS:
        nc.sync.dma_start(out=xx[:, :, s0:s0 + sw], in_=t_xx[:, :, s0:s0 + sw])
    # pin the exp table set now; input is a self-zeroed scratch (no DMA dep)
    nc.scalar.memzero(scr)
    nc.scalar.activation(scr, scr, mybir.ActivationFunctionType.Exp)
    nc.scalar.dma_start(out=wot_sb, in_=t_wot)
    # dead K halves + VT ones-columns; GpSimd is idle at startup
    nc.gpsimd.memset(kz0[HD:P, :], 0.0)
    nc.gpsimd.memset(kz1[0:HD, :], 0.0)
    nc.gpsimd.memset(vt_sb[:, :, HD:HD + 1], 1.0)
    nc.gpsimd.memset(vt_sb[:, :, 129:130], 1.0)

    ps = tc.alloc_tile_pool(name="ps", bufs=2, space="PSUM")
    ex_pool = tc.alloc_tile_pool(name="ex_sb", bufs=4)
    nrm = tc.alloc_tile_pool(name="nrm", bufs=2)

    def k_chunk(ci):
        s0, sw = S_CHUNKS[ci]
        psn = ps.tile([P, GRP * 512], F32, tag="sc", name="kps")[:, :sw]
        nc.tensor.matmul(psn, wqkv_sb[:, WK0, :], xx[:, 0, s0:s0 + sw],
                         start=True, stop=False)
        nc.tensor.matmul(psn, wqkv_sb[:, WK1, :], xx[:, 1, s0:s0 + sw],
                         start=False, stop=True)
        nc.vector.tensor_scalar_add(kz0[0:HD, s0:s0 + sw], psn[0:HD, :],
                                    bk_sb[0:HD, :])
        nc.vector.tensor_scalar_add(kz1[HD:P, s0:s0 + sw], psn[HD:P, :],
                                    bk_sb[HD:P, :])

    def k_tail():
        # chunks 2..4 (s 1024:2304, 1280 wide) in one PSUM alloc, one add pair
        psn = ps.tile([P, GRP * 512], F32, tag="sc", name="kps")[:, :1280]
        for ci in (2, 3, 4):
            s0, sw = S_CHUNKS[ci]
            o = s0 - 1024
            nc.tensor.matmul(psn[:, o:o + sw], wqkv_sb[:, WK0, :],
                             xx[:, 0, s0:s0 + sw], start=True, stop=False)
            nc.tensor.matmul(psn[:, o:o + sw], wqkv_sb[:, WK1, :],
                             xx[:, 1, s0:s0 + sw], start=False, stop=True)
        nc.vector.tensor_scalar_add(kz0[0:HD, 1024:S], psn[0:HD, :],
                                    bk_sb[0:HD, :])
        nc.vector.tensor_scalar_add(kz1[HD:P, 1024:S], psn[HD:P, :],
                                    bk_sb[HD:P, :])

    def q_multi(cis):
        s0 = S_CHUNKS[cis[0]][0]
        wtot = sum(S_CHUNKS[ci][1] for ci in cis)
        psn = ps.tile([P, GRP * 512], F32, tag="sc", name="qps")[:, :wtot]
        for ci in cis:
            c0, cw = S_CHUNKS[ci]
            o = c0 - s0
            nc.tensor.matmul(psn[:, o:o + cw], wqkv_sb[:, WQ0, :],
                             xx[:, 0, c0:c0 + cw], start=True, stop=False)
            nc.tensor.matmul(psn[:, o:o + cw], wqkv_sb[:, WQ1, :],
                             xx[:, 1, c0:c0 + cw], start=False, stop=True)
        nc.vector.tensor_scalar_add(q_sb[:, s0:s0 + wtot], psn, bq_sb)

    def vt_multi(base, n):
        # n consecutive t-tiles in one PSUM alloc, one grouped bias add
        psn = ps.tile([P, GRP * 512], F32, tag="sc", name="vtps")
        ps3 = psn[:, :n * P].rearrange("p (n d) -> p n d", d=P)
        for j in range(n):
            tt = base + j
            nc.tensor.matmul(ps3[:, j, :], xx[:, 0, tt * P:(tt + 1) * P],
                             wqkv_sb[:, WV0, :], start=True, stop=False)
            nc.tensor.matmul(ps3[:, j, :], xx[:, 1, tt * P:(tt + 1) * P],
                             wqkv_sb[:, WV1, :], start=False, stop=True)
        # vt cols per tt: [dA 0:64 | one | dB 65:129 | one]; write both halves
        va = vt_sb[:, base:base + n, 0:HD]
        vb = vt_sb[:, base:base + n, HD + 1:129]
        pa = bass.AP(tensor=ps3.tensor, offset=ps3.offset,
                     ap=[ps3.ap[0], ps3.ap[1], [ps3.ap[2][0], HD]])
        pb_src = ps3[:, :, HD:P]
        bva = bass.AP(tensor=bv_bc.tensor, offset=bv_bc.offset,
                      ap=[bv_bc.ap[0], [0, n], [bv_bc.ap[1][0], HD]])
        bvb_base = bv_bc[:, HD:P]
        bvb = bass.AP(tensor=bvb_base.tensor, offset=bvb_base.offset,
                      ap=[bvb_base.ap[0], [0, n], bvb_base.ap[1]])
        nc.vector.tensor_add(va, pa, bva)
        nc.vector.tensor_add(vb, pb_src, bvb)

    def emit_av(pend):
        ex, g, ot, h, sw = pend
        for j in range(GRP):
            tt = g * GRP + j
            nc.tensor.matmul(ot, vt_sb[:, tt, h * 65:(h + 1) * 65],
                             ex[:, j * sw:(j + 1) * sw],
                             start=(tt == 0), stop=(tt == TT - 1))

    def wo_chunk(ci):
        s0, sw = S_CHUNKS[ci]
        psn = ps.tile([P, GRP * 512], F32, tag="sc", name="wops")
        for half in range(2):
            pw = psn[:, half * 512:half * 512 + sw]
            nc.tensor.matmul(pw, wot_sb[:, half * P:(half + 1) * P],
                             attn_full[:, s0:s0 + sw], start=True, stop=True)
            nc.vector.tensor_copy(ob[:, half, s0:s0 + sw], pw)

    def out_dma(s0, sw):
        for half in range(2):
            eng = nc.sync if half == 0 else nc.gpsimd
            eng.dma_start(out=t_out[half, :, s0:s0 + sw],
                          in_=ob[:, half, s0:s0 + sw])

    def emit_norm(ot, h, s0, sw):
        comb = nrm.tile([65, 512], F32, tag="comb", name="comb")[:, :sw]
        nc.vector.tensor_copy(comb, ot)
        # row-sum lives on partition 64; recip/broadcast need base partition 0
        rs0 = nrm.tile([1, 512], F32, tag="rs0", name="rs0")[:, :sw]
        nc.sync.dma_start(out=rs0, in_=comb[HD:HD + 1, :])
        rinv = nrm.tile([1, 512], F32, tag="rinv", name="rinv")[:, :sw]
        nc.vector.reciprocal_approx_fast(rinv, rs0)
        rb = nrm.tile([HD, 512], F32, tag="rb", name="rb")[:, :sw]
        nc.gpsimd.partition_broadcast(rb, rinv)
        if h == 0:
            nc.vector.tensor_mul(attn_full[0:HD, s0:s0 + sw], comb[0:HD, :], rb)
        else:
            a1 = nrm.tile([HD, 512], BF16, tag="a1", name="a1")[:, :sw]
            nc.vector.tensor_mul(a1, comb[0:HD, :], rb)
            nc.sync.dma_start(out=attn_full[HD:P, s0:s0 + sw], in_=a1)

    # ---- prologue compute: first two K chunks + Q chunk 0 ----
    k_chunk(0)
    q_multi([0])
    k_chunk(1)

    # ---- attention: software-pipelined across all (s-chunk, head) units ----
    pend = None       # (ex, g, ot, h, sw): exp batch whose attn@V is pending
    pend_norm = None  # (ot, h, s0, sw, ci): unit awaiting normalization
    wo_q = []         # (unit_idx, ci) chunks whose Wo projection is pending
    unit = 0
    for ci, (s0, sw) in enumerate(S_CHUNKS):
        for h in range(2):
            unit += 1
            kz = kz0 if h == 0 else kz1
            ot = ps.tile([65, 512], F32, tag="ot", name="ot")[:, :sw]
            for g in range(NG):
                sc = ps.tile([P, GRP * 512], F32, tag="sc", name="sc")[:, :GRP * sw]
                for j in range(GRP):
                    tt = g * GRP + j
                    nc.tensor.matmul(sc[:, j * sw:(j + 1) * sw],
                                     kz[:, tt * P:(tt + 1) * P],
                                     q_sb[:, s0:s0 + sw],
                                     start=True, stop=True)
                # startup weave: VT + remaining Q between early exp batches;
                # VT batch b lands just before the attn@V that reads it
                if unit == 1:
                    if g == 1:
                        k_tail()  # before scores g2 (t 768:1152 needs chunk 2)
                        vt_multi(0, 6)
                    elif g == 3:
                        vt_multi(6, 6)
                        q_multi([1])
                    elif g == 5:
                        vt_multi(12, 6)
                        q_multi([2])
                elif unit == 2 and g == 0:
                    q_multi([3])
                    q_multi([4])
                if pend is not None:
                    emit_av(pend)
                    if pend[1] == NG - 1:  # last batch of its unit
                        emit_norm(*pend_norm[:4])
                ex = ex_pool.tile([P, GRP * 512], BF16, tag="ex", name="ex")[:, :GRP * sw]
                nc.scalar.activation(ex, sc, mybir.ActivationFunctionType.Exp,
                                     scale=SCALE)
                pend = (ex, g, ot, h, sw)
                if g == NG - 1:
                    pend_norm = (ot, h, s0, sw, ci)
    emit_av(pend)
    emit_norm(*pend_norm[:4])
    for ci_w in range(len(S_CHUNKS)):
        wo_chunk(ci_w)
        if ci_w == 1:
            out_dma(0, 1024)
    out_dma(1024, 1280)

    nrm.release()
    ex_pool.release()
    ps.release()
    singles.release()


_NC_CACHE = {}


def build_nc():
    if "nc" not in _NC_CACHE:
        nc = bacc.Bacc("TRN2", target_bir_lowering=False, debug=False, num_devices=8)
        with tile.TileContext(nc) as tc:
            _body(tc)
        nc.compile()
        _NC_CACHE["nc"] = nc
    return _NC_CACHE["nc"]


def make_in_maps(x, Wq, bq, Wk, bk, Wv, bv, Wo, bo):
    import ml_dtypes
    bf16 = ml_dtypes.bfloat16
    N = x.shape[0]
    # (N, C, S) -> per batch (P, 2, S): partition p holds rows p and p+128
    xf = np.asarray(x, np.float32).reshape(N, C, S).reshape(N, 2, P, S)
    xf = np.ascontiguousarray(xf.transpose(0, 2, 1, 3).astype(bf16))
    in_maps = []
    for c in range(8):
        n, hp = c // 2, c % 2
        ch = slice(hp * P, (hp + 1) * P)
        wqkv = np.empty((P, 6, P), np.float32)
        for i, W in enumerate((Wq, Wk, Wv)):
            wt = np.asarray(W, np.float32)[ch].T  # (C, 128): [c_in, d_out]
            wqkv[:, 2 * i, :] = wt[0:P]
            wqkv[:, 2 * i + 1, :] = wt[P:C]
        wot = np.asarray(Wo, np.float32)[:, ch].T  # (128, 256)
        bvv = np.asarray(bv, np.float32)[ch]
        in_maps.append({
            "xx": xf[n],
            "wqkv": np.ascontiguousarray(wqkv.astype(bf16)),
            "wot": np.ascontiguousarray(wot.astype(bf16)),
            "bq": np.ascontiguousarray(np.asarray(bq, np.float32)[ch].reshape(P, 1)),
            "bk": np.ascontiguousarray(np.asarray(bk, np.float32)[ch].reshape(P, 1)),
            "bv": np.ascontiguousarray(np.broadcast_to(bvv[None, :], (P, P))),
        })
    return in_maps


def run(inputs, **kwargs):
    """Run on 8 cores; returns (full output, BassKernelResults)."""
    nc = build_nc()
    in_maps = make_in_maps(**inputs)
    res = run_bass_kernel_spmd(nc, in_maps, core_ids=list(range(8)), **kwargs)
    x = np.asarray(inputs["x"], np.float32)
    bo = np.asarray(inputs["bo"], np.float32)
    N, _, H, W = x.shape
    out = np.empty((N, C, S), np.float32)
    for n in range(N):
        p0 = np.asarray(res.results[2 * n]["out"], np.float32).reshape(C, S)
        p1 = np.asarray(res.results[2 * n + 1]["out"], np.float32).reshape(C, S)
        out[n] = x[n].reshape(C, S) + p0 + p1 + bo[:, None]
    return out.reshape(N, C, H, W), res


def kernel(**inputs):
    out, _ = run(inputs)
    return out
